# revision 8
# baseline (speedup 1.0000x reference)
"""NodeEquilibriumLoss Trainium2 kernel.

residual[b] = (EA[b] * e[b]) @ S - q[b] - r[b];  out = mean(residual^2)

S[elem, 2*node+c] = sum_k [elem_ids[k]==elem][node_ids[k]==node] * vecs[k, c]
is the fixed sparse linear map implementing the reference's gather+scatter-add.

Sharding: data-parallel over batch, 8 cores x 512 rows. Per core:
  - S (bf16, [2048, 2048]) is built ON DEVICE from ~512KB of compact
    (index, value) tables via gpsimd.local_scatter (64 calls), avoiding an
    8MB/core HBM load.
  - per 128-row batch tile: axial = EA*e (DVE, bf16 out), one fused DMA-xbar
    transpose [128,2048] -> [128,16,128] putting elem on partitions, 64 bf16
    matmuls accumulating K=2048 in PSUM, d = psum - (q+r), then per-partition
    sum(d^2) via scalar_tensor_tensor.
  - output: [128, 16] per-partition partial sums; host reduces in fp64.
"""

import numpy as np
import ml_dtypes

B, NE, NN, E2 = 4096, 2048, 1024, 4096
N2 = 2 * NN
NCORES = 8
SHARD = B // NCORES   # 512
BT = 128              # batch rows per tile
NT = SHARD // BT      # 4 batch tiles per core
KT = NE // 128        # 16 contraction tiles
NBLK = 4              # output column blocks of 512
NFREE = N2 // NBLK    # 512
NHALF = 4             # scatter chunks per k-tile (512 cols each)
NIDX = 16             # padded nonzeros per (elem row, chunk)

_CACHE = {}


def _build_bass():
    from concourse import bacc
    import concourse.mybir as mybir
    import concourse.tile as tile

    f32 = mybir.dt.float32
    bf16 = mybir.dt.bfloat16
    i16 = mybir.dt.int16
    mult = mybir.AluOpType.mult

    nc = bacc.Bacc("TRN2", target_bir_lowering=False, debug=False,
                   num_devices=NCORES)
    EA = nc.dram_tensor("EA", [SHARD, NE], f32, kind="ExternalInput").ap()
    ee = nc.dram_tensor("e", [SHARD, NE], f32, kind="ExternalInput").ap()
    qq = nc.dram_tensor("q", [SHARD, N2], f32, kind="ExternalInput").ap()
    rr = nc.dram_tensor("r", [SHARD, N2], f32, kind="ExternalInput").ap()
    sidx = nc.dram_tensor("sidx", [128, KT, NHALF, NIDX], i16,
                          kind="ExternalInput").ap()
    sval = nc.dram_tensor("sval", [128, KT, NHALF, NIDX], bf16,
                          kind="ExternalInput").ap()
    out = nc.dram_tensor("out", [128, NT * NBLK], f32,
                         kind="ExternalOutput").ap()

    with tile.TileContext(nc) as tc:
        with (
            tc.tile_pool(name="sconst", bufs=1) as sconst,
            tc.tile_pool(name="io", bufs=2) as io,
            tc.tile_pool(name="work", bufs=2) as work,
            tc.tile_pool(name="ps", bufs=4, space="PSUM") as psp,
        ):
            # --- build S in SBUF from compact scatter tables ---
            idx_t = sconst.tile([128, KT, NHALF, NIDX], i16)
            val_t = sconst.tile([128, KT, NHALF, NIDX], bf16)
            nc.sync.dma_start(out=idx_t, in_=sidx)
            nc.sync.dma_start(out=val_t, in_=sval)
            S_tiles = {}
            # h-outer: chunk h feeds output block nb=h, so the first matmul
            # group only waits for the first quarter of the build
            for h in range(NHALF):
                for kt in range(KT):
                    st = sconst.tile([128, NE // NHALF], bf16,
                                     tag=f"S_{kt}_{h}")
                    nc.gpsimd.local_scatter(
                        out_ap=st[:, :], data_ap=val_t[:, kt, h, :],
                        idxs_ap=idx_t[:, kt, h, :],
                        channels=128, num_elems=N2 // NHALF, num_idxs=NIDX,
                    )
                    S_tiles[(kt, h)] = st

            acc = sconst.tile([128, NT * NBLK], f32)

            for it in range(NT):
                sl = slice(it * BT, (it + 1) * BT)
                ea_t = io.tile([128, NE], f32, tag="ea")
                e_t = io.tile([128, NE], f32, tag="e")
                nc.sync.dma_start(out=ea_t, in_=EA[sl, :])
                nc.sync.dma_start(out=e_t, in_=ee[sl, :])

                ax = work.tile([128, NE], bf16, tag="ax")
                nc.vector.tensor_mul(ax, ea_t, e_t)

                # fused xbar transpose: axT[p, kt, b] = ax[b, kt*128+p]
                axT = work.tile([128, KT, 128], bf16, tag="axT")
                nc.scalar.dma_start_transpose(axT[:], ax[:])

                q_t = io.tile([128, N2], f32, tag="q")
                r_t = io.tile([128, N2], f32, tag="r")
                nc.scalar.dma_start(out=q_t, in_=qq[sl, :])
                nc.scalar.dma_start(out=r_t, in_=rr[sl, :])
                t_t = work.tile([128, N2], f32, tag="t")
                nc.vector.tensor_add(t_t, q_t, r_t)

                d_t = work.tile([128, N2], f32, tag="d")
                for nb in range(NBLK):
                    ps = psp.tile([128, NFREE], f32, tag="ps")
                    h, off = divmod(nb * NFREE, NE // NHALF)
                    for kt in range(KT):
                        nc.tensor.matmul(
                            ps,
                            lhsT=axT[:, kt, :],
                            rhs=S_tiles[(kt, h)][:, off:off + NFREE],
                            start=(kt == 0),
                            stop=(kt == KT - 1),
                        )
                    dn = d_t[:, nb * NFREE:(nb + 1) * NFREE]
                    nc.vector.tensor_sub(
                        dn, ps, t_t[:, nb * NFREE:(nb + 1) * NFREE])
                    col = it * NBLK + nb
                    nc.vector.scalar_tensor_tensor(
                        out=dn, in0=dn, scalar=1.0, in1=dn,
                        op0=mult, op1=mult,
                        accum_out=acc[:, col:col + 1],
                    )

            nc.sync.dma_start(out=out, in_=acc)

    nc.compile()
    return nc


def _get_bass():
    if "nc" not in _CACHE:
        _CACHE["nc"] = _build_bass()
    return _CACHE["nc"]


def _build_tables(vecs, node_ids, elem_ids):
    """Compact per-(elem-row, half) scatter tables for local_scatter."""
    half_w = N2 // NHALF
    buckets = {}
    for k in range(E2):
        e_row = int(elem_ids[k])
        for c in (0, 1):
            col = 2 * int(node_ids[k]) + c
            h, local = divmod(col, half_w)
            key = (e_row, h)
            d = buckets.setdefault(key, {})
            d[local] = d.get(local, 0.0) + float(vecs[k, c])
    sidx = np.full((128, KT, NHALF, NIDX), -1, dtype=np.int16)
    sval = np.zeros((128, KT, NHALF, NIDX), dtype=np.float32)
    for (e_row, h), d in buckets.items():
        kt, p = divmod(e_row, 128)
        items = list(d.items())
        assert len(items) <= NIDX, f"bucket overflow: {len(items)} > {NIDX}"
        for j, (local, v) in enumerate(items):
            sidx[p, kt, h, j] = local
            sval[p, kt, h, j] = v
    return sidx, sval.astype(ml_dtypes.bfloat16)


def _prep_in_maps(EA, e, q, r, vecs, node_ids, elem_ids):
    EA = np.ascontiguousarray(np.asarray(EA, dtype=np.float32))
    e = np.ascontiguousarray(np.asarray(e, dtype=np.float32))
    q = np.ascontiguousarray(np.asarray(q, dtype=np.float32)).reshape(B, N2)
    r = np.ascontiguousarray(np.asarray(r, dtype=np.float32)).reshape(B, N2)
    vecs = np.asarray(vecs, dtype=np.float32)
    sidx, sval = _build_tables(vecs, np.asarray(node_ids), np.asarray(elem_ids))

    in_maps = []
    for c in range(NCORES):
        sl = slice(c * SHARD, (c + 1) * SHARD)
        in_maps.append({
            "EA": EA[sl], "e": e[sl], "q": q[sl], "r": r[sl],
            "sidx": sidx, "sval": sval,
        })
    return in_maps


def _reduce_outs(results):
    total = 0.0
    for c in range(NCORES):
        total += results[c]["out"].astype(np.float64).sum()
    return np.array(total / (B * NN * 2), dtype=np.float32)


def kernel_run(EA, e, q, r, vecs, node_ids, elem_ids, trace=False):
    from concourse.bass_utils import run_bass_kernel_spmd

    nc = _get_bass()
    in_maps = _prep_in_maps(EA, e, q, r, vecs, node_ids, elem_ids)
    res = run_bass_kernel_spmd(nc, in_maps, core_ids=list(range(NCORES)),
                               trace=trace)
    return _reduce_outs(res.results), res


def kernel(EA, e, q, r, vecs, node_ids, elem_ids):
    val, _ = kernel_run(EA, e, q, r, vecs, node_ids, elem_ids, trace=False)
    return val
